# revision 3
# baseline (speedup 1.0000x reference)
"""Distributed causal multi-head attention for Trainium2 (8 NeuronCores), v2.

Problem: B=2, S=2048, d_model=1024, 16 heads x 64 dims, causal softmax attention.

Strategy v2 (DP over batch x TP over 4 heads; host-side reduction):
  - Core c = (b, g): batch b = c//4, head-group g = c%4 owns heads 4g..4g+3
    (256 of the 1024 QKV features). Input per core: x[b] (4 MB bf16) +
    weights slice (2.2 MB) vs 9.6 MB in v1; output partial 4 MB vs 8 MB.
    DMA is the shared bottleneck (~120-150 GB/s/core under 8-core
    contention), so halving bytes shrinks the DMA-gated head and tail.
  - Within a core, the 4 heads are processed as 2 head-pairs p in {0,1}
    using the same S^T-layout attention machinery as v1: scores^T = K^T x Q^T
    with the pair's two K=64 matmuls row-tiled onto disjoint PE row groups,
    exp on ScalarE, causal triu mask multiply on the diagonal tiles, AV
    accumulation in PSUM with 64 appended ones columns per head producing the
    softmax denominator replicated in PSUM partitions 64-127 for free.
  - V projection computes all 4 heads at once (moving N=256 vs 128 in v1,
    halving its LDWEIGHTS-bound cost); output tile layout per k-tile is
    [v_h0|ones|v_h1|ones|v_h2|ones|v_h3|ones] written with strided APs
    (one bias-add + one memset per k-tile).
  - Chunk order (q0,p0),(q0,p1),(q1,p0),(q1,p1),... matches DMA arrival
    (chunk qc only needs x-chunks 0..qc); output projection of q-chunk qc
    runs as filler inside later chunks; the q3 output projection pipelines
    per-rt into the last chunk's normalization (short exposed tail).
  - Output is packed [128, 4*4096] (contiguous 8 KB per-partition rows ->
    efficient DMA packets); the host unpacks and sums the 4 partials/batch.
  - Dummy warmup matmuls keep the PE HAM activity monitor from throttling
    the clock during the DMA-gated ramp.
"""
import os
import sys

sys.path.insert(0, "/opt/trn_rl_repo")

import numpy as np
import ml_dtypes

from concourse import bacc, mybir, tile
from concourse.ap import AP
from concourse.bass_utils import run_bass_kernel_spmd

BF16 = mybir.dt.bfloat16
F32 = mybir.dt.float32

B, S, DM = 2, 2048, 1024
H, DK = 16, 64
N_CORES = 8
FPC = 256            # features per core = 4 heads x 64
NRC = 4              # 512-token chunks per batch
NKT = S // 128       # k-tiles per batch = 16
SCALE = 1.0 / 8.0    # 1/sqrt(64)
# wpk cols: wq_f0 (1024) | wk_f0 (1024) | wv (2048) | wq_f1 (1024) | wk_f1
#           (1024) | wo_p0 (1024) | wo_p1 (1024) | mask (128) | bvb (256)
# DMA pieces: [0:4096] (all chunk-A needs), [4096:6144], [6144:8576]
WQ0_O, WK0_O, WV_O, WQ1_O, WK1_O, WO_O = 0, 1024, 2048, 4096, 5120, 6144
MASK_O, BVB_O = 8192, 8320
WPK_N = 8576
N_DUMMY = 27

_cache = {}


def _blocked(ap_src, nblk, blk_stride, blk_w, col_off=0):
    """AP selecting nblk blocks of blk_w cols spaced blk_stride, from a 2D AP."""
    t = ap_src
    p = list(t.ap)
    # p = [[pstride, np], [1, ncols]]
    return AP(t.tensor, t.offset + col_off, [list(p[0]), [blk_stride, nblk], [1, blk_w]])


def _build():
    nc = bacc.Bacc("TRN2", target_bir_lowering=False, debug=False, num_devices=N_CORES)

    # xtr[p, rc*4096 + kc*512 + j] = x[b]^T[kc*128+p, rc*512+j]
    xtr = nc.dram_tensor("xtr", [128, NRC * 4096], BF16, kind="ExternalInput")
    wpk = nc.dram_tensor("wpk", [128, WPK_N], BF16, kind="ExternalInput")
    bpk = nc.dram_tensor("bpk", [128, 4], F32, kind="ExternalInput")
    # out[p, qc*4096 + rt*1024 + d] = partial_out[qc*512 + rt*128 + p, d]
    out_ext = nc.dram_tensor("out", [128, NRC * 4096], BF16, kind="ExternalOutput")

    EXP = mybir.ActivationFunctionType.Exp
    IDENT = mybir.ActivationFunctionType.Identity

    with tile.TileContext(nc) as tc:
        with (
            tc.tile_pool(name="xtp", bufs=1) as xtp,
            tc.tile_pool(name="wts", bufs=1) as wts,
            tc.tile_pool(name="qkv", bufs=1) as qkvp,
            tc.tile_pool(name="vnp", bufs=1) as vnp,
            tc.tile_pool(name="pp", bufs=4) as pp,
            tc.tile_pool(name="den", bufs=2) as denp,
            tc.tile_pool(name="otp", bufs=1) as otp,
            tc.tile_pool(name="outp", bufs=2) as outp,
            tc.tile_pool(name="psmm", bufs=2, space="PSUM") as psmm,
            tc.tile_pool(name="psS", bufs=2, space="PSUM") as psS,
            tc.tile_pool(name="psO", bufs=1, space="PSUM") as psO,
        ):
            # ---------- dummy warmup (PE busy during input DMA => HAM hot) ----
            dummy_sb = wts.tile([128, 512], BF16, tag="dum", name="dummy_sb")
            nc.vector.memset(dummy_sb[:], 0.0)

            def dummy(i, n=512):
                ps = psS.tile([128, 1024], F32, tag="s", name=f"dum{i}")
                nc.tensor.matmul(
                    ps[:, 0:n], dummy_sb[:, 0:128], dummy_sb[:, 0:n],
                    start=True, stop=True,
                )

            for i in range(N_DUMMY):
                dummy(i)

            # ---------- input DMAs, in consumption order ----------
            wpk_sb = wts.tile([128, WPK_N], BF16, tag="wpk", name="wpk_sb")
            xt_all = xtp.tile([128, NRC * 4096], BF16, tag="xt", name="xt_all")
            bpk_sb = wts.tile([128, 4], F32, tag="bpk", name="bpk_sb")
            nc.sync.dma_start(wpk_sb[:, 0:WQ1_O], wpk[:, 0:WQ1_O])  # qk f0 + wv
            nc.sync.dma_start(bpk_sb[:], bpk[:])
            nc.sync.dma_start(xt_all[:, 0:4096], xtr[:, 0:4096])    # rc0
            nc.sync.dma_start(wpk_sb[:, WQ1_O:WO_O], wpk[:, WQ1_O:WO_O])
            nc.sync.dma_start(
                xt_all[:, 4096:2 * 4096], xtr[:, 4096:2 * 4096]     # rc1
            )
            nc.sync.dma_start(wpk_sb[:, WO_O:WPK_N], wpk[:, WO_O:WPK_N])
            for rc in range(2, NRC):
                nc.sync.dma_start(
                    xt_all[:, rc * 4096:(rc + 1) * 4096],
                    xtr[:, rc * 4096:(rc + 1) * 4096],
                )

            _wbase = {("q", 0): WQ0_O, ("k", 0): WK0_O, ("q", 1): WQ1_O, ("k", 1): WK1_O}

            def wqk_sl(which, kc, fc):   # [128, 128] stationary
                o = _wbase[(which, fc)] + kc * 128
                return wpk_sb[:, o:o + 128]

            def wv_sl(kc):               # [128, 256] moving
                o = WV_O + kc * 256
                return wpk_sb[:, o:o + 256]

            wo_sb = [wpk_sb[:, WO_O:WO_O + DM], wpk_sb[:, WO_O + DM:WO_O + 2 * DM]]
            mask_sb = wpk_sb[:, MASK_O:MASK_O + 128]
            bvb_sb = wpk_sb[:, BVB_O:BVB_O + 256]
            b_col = {("q", 0): 0, ("q", 1): 1, ("k", 0): 2, ("k", 1): 3}

            # qT/kT per head-pair: [128 (2 heads x 64), 2048 tokens]
            qT = [qkvp.tile([128, S], BF16, tag=f"qT{p}", name=f"qT{p}") for p in (0, 1)]
            kT = [qkvp.tile([128, S], BF16, tag=f"kT{p}", name=f"kT{p}") for p in (0, 1)]
            # v natural layout per k-tile: [128 tok, v0|1s|v1|1s|v2|1s|v3|1s]
            v_nat = [None] * NKT
            ot_st = {}
            osb_st = {}

            def xsl(rc, kc, lo, n):
                o = rc * 4096 + kc * 512 + lo
                return xt_all[:, o:o + n]

            # ---------- filler units ----------
            def unit_proj(rc, which, fc):
                def emit():
                    ps = psmm.tile([128, 512], F32, tag="mm", name=f"ps_{which}{fc}_{rc}")
                    for kc in range(8):
                        nc.tensor.matmul(
                            ps[:], wqk_sl(which, kc, fc), xsl(rc, kc, 0, 512),
                            start=(kc == 0), stop=(kc == 7),
                        )
                    dst = (qT if which == "q" else kT)[fc]
                    nc.scalar.activation(
                        dst[:, rc * 512:(rc + 1) * 512], ps[:], IDENT,
                        bias=bpk_sb[:, b_col[(which, fc)]:b_col[(which, fc)] + 1],
                    )
                return emit

            def unit_vtr(rc, i):
                def emit():
                    kt = rc * 4 + i
                    ps = psmm.tile([128, 512], F32, tag="mm", name=f"psv{rc}_{i}")
                    for kc in range(8):
                        nc.tensor.matmul(
                            ps[:, 0:256], xsl(rc, kc, i * 128, 128), wv_sl(kc),
                            start=(kc == 0), stop=(kc == 7),
                        )
                    vn = vnp.tile([128, 512], BF16, tag=f"vn{kt}", name=f"vn{kt}")
                    # vn[:, 128h:128h+64] = ps[:, 64h:64h+64] + bvb[:, 64h:64h+64]
                    nc.vector.tensor_add(
                        _blocked(vn[:], 4, 128, 64),
                        _blocked(ps[:], 4, 64, 64),
                        bvb_sb[:, 0:256],
                    )
                    nc.vector.memset(_blocked(vn[:], 4, 128, 64, col_off=64), 1.0)
                    v_nat[kt] = vn
                return emit

            def unit_outproj(qc, rt, nci):
                def emit():
                    ps = psmm.tile([128, 512], F32, tag="mm", name=f"pso{qc}_{rt}_{nci}")
                    for p in (0, 1):
                        lh = ot_st[(p, qc)][:, rt * 128:(rt + 1) * 128]
                        nc.tensor.matmul(
                            ps[:], lh, wo_sb[p][:, nci * 512:(nci + 1) * 512],
                            start=(p == 0), stop=(p == 1),
                        )
                    osb = osb_st[qc]
                    dst = osb[:, rt * 1024 + nci * 512: rt * 1024 + (nci + 1) * 512]
                    if qc == 3:
                        nc.scalar.copy(dst, ps[:])
                    else:
                        nc.vector.tensor_copy(dst, ps[:])
                    if rt == 3 and nci == 1 and qc < 3:
                        nc.sync.dma_start(
                            out_ext[:, qc * 4096:(qc + 1) * 4096], osb[:]
                        )
                return emit

            def new_osb(qc):
                osb = outp.tile([128, 4096], BF16, tag="ob", name=f"osb{qc}")
                osb_st[qc] = osb

            # ---------- attention chunk ----------
            def chunk(p, qc, fillers, pre_av=None, last=False):
                nkt = 4 * qc + 4
                q0 = qc * 512
                fillers = list(fillers)[::-1]
                n_fill = len(fillers)

                o_ps = [
                    psO.tile([128, 512], F32, tag=f"o{h}", name=f"o_ps{h}_{p}_{qc}")
                    for h in (0, 1)
                ]

                def emit_s(kt):
                    lo = max(0, 128 * (kt - 4 * qc))
                    s_ps = psS.tile([128, 1024], F32, tag="s", name=f"s_{p}_{qc}_{kt}")
                    k_sl = slice(kt * 128, (kt + 1) * 128)
                    for h in (0, 1):
                        hp = slice(64 * h, 64 * h + 64)
                        nc.tensor.matmul(
                            s_ps[:, 512 * h + lo:512 * h + 512],
                            kT[p][hp, k_sl], qT[p][hp, q0 + lo:q0 + 512],
                            start=True, stop=True,
                        )
                    return s_ps, lo

                def emit_exp_av(kt, s_ps, lo):
                    p_sb = pp.tile([128, 1024], BF16, tag="p", name=f"p_{p}_{qc}_{kt}")
                    if lo == 0:
                        nc.scalar.activation(p_sb[:], s_ps[:], EXP, scale=SCALE)
                    elif lo <= 256:
                        # one contiguous ACTIVATE spanning both heads' valid
                        # ranges plus the dead gap [512:512+lo] (written but
                        # never read by mask or AV): cheaper than two ops
                        # while 1024-lo < 2*(512-lo) + ACT fixed cost
                        nc.scalar.activation(
                            p_sb[:, lo:1024], s_ps[:, lo:1024], EXP, scale=SCALE
                        )
                    else:
                        for h in (0, 1):
                            nc.scalar.activation(
                                p_sb[:, 512 * h + lo:512 * h + 512],
                                s_ps[:, 512 * h + lo:512 * h + 512],
                                EXP, scale=SCALE,
                            )
                    d = 128 * (kt - 4 * qc)
                    if d >= 0:
                        hi = min(512, d + 128)
                        for h in (0, 1):
                            nc.vector.tensor_mul(
                                p_sb[:, 512 * h + lo:512 * h + hi],
                                p_sb[:, 512 * h + lo:512 * h + hi],
                                mask_sb[:, 0:hi - lo],
                            )
                    for h in (0, 1):
                        nc.tensor.matmul(
                            o_ps[h][:, lo:512],
                            v_nat[kt][:, 128 * (2 * p + h):128 * (2 * p + h) + 128],
                            p_sb[:, 512 * h + lo:512 * h + 512],
                            start=(kt == 0), stop=(kt == nkt - 1),
                        )

                popped = 0
                s_cur = None
                for kt in range(nkt):
                    if pre_av is not None and kt in pre_av:
                        if s_cur is None:
                            s_cur = emit_s(kt)
                        pre_av[kt]()
                    if s_cur is None:
                        s_cur = emit_s(kt)
                    s_nxt = emit_s(kt + 1) if kt + 1 < nkt else None
                    # one filler early: covers the exp latency of the first
                    # k-tile before any AV work is ready for the PE
                    if kt == 0 and pre_av is None and fillers:
                        fillers.pop()()
                        popped += 1
                    emit_exp_av(kt, *s_cur)
                    s_cur = s_nxt
                    spread = nkt + 4 if last else nkt
                    want = (kt + 1) * n_fill // spread
                    while fillers and popped < want:
                        fillers.pop()()
                        popped += 1

                # ---- normalization (denominator in psum partitions 64-127) --
                ot = otp.tile([128, 512], BF16, tag=f"ot{p}_{qc}", name=f"ot{p}_{qc}")
                ot_st[(p, qc)] = ot
                rcp = [None, None]
                if not last:
                    for h in (0, 1):
                        rcp[h] = denp.tile([64, 512], F32, tag=f"d{h}", name=f"d{h}_{p}{qc}")
                        nc.vector.tensor_copy(rcp[h][0:64, :], o_ps[h][64:128, :])
                        nc.vector.reciprocal_approx_fast(rcp[h][0:64, :], rcp[h][0:64, :])
                        nc.vector.tensor_mul(
                            ot[64 * h:64 * h + 64, :], o_ps[h][0:64, :], rcp[h][0:64, :]
                        )
                    while fillers:
                        fillers.pop()()
                else:
                    # last chunk: per-rt normalization pipelined into the q3
                    # output projection, with dummies keeping the PE warm.
                    new_osb(3)
                    td = [0]

                    def tail_dummy(n=256):
                        dummy(f"t{td[0]}", n=n)
                        td[0] += 1

                    for h in (0, 1):
                        rcp[h] = denp.tile([64, 512], F32, tag=f"d{h}", name=f"d{h}_t")
                        nc.vector.tensor_copy(rcp[h][0:64, :], o_ps[h][64:128, :])
                        nc.vector.reciprocal_approx_fast(rcp[h][0:64, :], rcp[h][0:64, :])
                    tail_dummy(512)
                    tail_dummy(512)
                    for rt in range(4):
                        csl = slice(rt * 128, (rt + 1) * 128)
                        for h in (0, 1):
                            nc.vector.tensor_mul(
                                ot[64 * h:64 * h + 64, csl],
                                o_ps[h][0:64, csl],
                                rcp[h][0:64, csl],
                            )
                        tail_dummy()
                        for nci in (0, 1):
                            unit_outproj(3, rt, nci)()
                            nc.sync.dma_start(
                                out_ext[:, 3 * 4096 + rt * 1024 + nci * 512:
                                        3 * 4096 + rt * 1024 + (nci + 1) * 512],
                                osb_st[3][:, rt * 1024 + nci * 512:
                                        rt * 1024 + (nci + 1) * 512],
                            )
                        if rt < 3:
                            tail_dummy()

            # ---------- schedule ----------
            # pre-A units: rc0 projections for pair 0 + V k-tiles 0..3
            unit_proj(0, "q", 0)()
            unit_proj(0, "k", 0)()

            # A = (q0, p0): V(0, i) interleaved before each AV
            chunk(0, 0,
                  fillers=[unit_proj(0, "k", 1), unit_proj(0, "q", 1)],
                  pre_av={i: unit_vtr(0, i) for i in range(4)})
            # B = (q0, p1)
            chunk(1, 0, fillers=[unit_proj(1, "q", 0), unit_proj(1, "k", 0)]
                  + [unit_vtr(1, i) for i in range(4)])
            new_osb(0)
            # C = (q1, p0)
            chunk(0, 1, fillers=[unit_proj(1, "q", 1), unit_proj(1, "k", 1)]
                  + [unit_outproj(0, rt, nci) for rt in range(4) for nci in (0, 1)])
            # D = (q1, p1)
            chunk(1, 1, fillers=[unit_proj(2, "q", 0), unit_proj(2, "k", 0)]
                  + [unit_vtr(2, i) for i in range(4)])
            new_osb(1)
            # E = (q2, p0)
            chunk(0, 2, fillers=[unit_proj(2, "q", 1), unit_proj(2, "k", 1)]
                  + [unit_outproj(1, rt, nci) for rt in range(4) for nci in (0, 1)])
            # F = (q2, p1)
            chunk(1, 2, fillers=[unit_proj(3, "q", 0), unit_proj(3, "k", 0)]
                  + [unit_vtr(3, i) for i in range(4)])
            new_osb(2)
            # G = (q3, p0)
            chunk(0, 3, fillers=[unit_proj(3, "q", 1), unit_proj(3, "k", 1)]
                  + [unit_outproj(2, rt, nci) for rt in range(4) for nci in (0, 1)])
            # H = (q3, p1): last; outproj(3) pipelined per-rt inside
            chunk(1, 3, fillers=[], last=True)

    nc.compile()
    return nc


def kernel(x, Wq, bq, Wk, bk, Wv, bv, Wo):
    if "nc" not in _cache:
        _cache["nc"] = _build()
    nc = _cache["nc"]

    bf = ml_dtypes.bfloat16
    x_f = np.asarray(x, np.float32)
    wo_f = np.asarray(Wo, np.float32)
    trimask = np.triu(np.ones((128, 128), np.float32))

    xtr_b = []
    for b in range(B):
        xT = x_f[b].T                                     # [1024, 2048]
        xtr_b.append(np.ascontiguousarray(
            xT.reshape(8, 128, NRC, 512).transpose(1, 2, 0, 3).reshape(128, NRC * 4096)
        ).astype(bf))

    in_maps = []
    for c in range(N_CORES):
        b, g = c // 4, c % 4
        sl = slice(g * FPC, (g + 1) * FPC)
        wpk_h = np.empty((128, WPK_N), np.float32)
        for (base0, base1), W in (((WQ0_O, WQ1_O), Wq), ((WK0_O, WK1_O), Wk)):
            Wc = np.asarray(W, np.float32)[:, sl]         # [1024, 256]
            for fc, base in ((0, base0), (1, base1)):
                wpk_h[:, base:base + 1024] = (
                    Wc[:, fc * 128:(fc + 1) * 128]
                    .reshape(8, 128, 128).transpose(1, 0, 2).reshape(128, 1024)
                )
        Wvc = np.asarray(Wv, np.float32)[:, sl]
        wpk_h[:, WV_O:WV_O + 2048] = (
            Wvc.reshape(8, 128, 256).transpose(1, 0, 2).reshape(128, 2048)
        )
        wpk_h[:, WO_O:WO_O + DM] = wo_f[sl][0:128, :]
        wpk_h[:, WO_O + DM:WO_O + 2 * DM] = wo_f[sl][128:256, :]
        wpk_h[:, MASK_O:MASK_O + 128] = trimask
        wpk_h[:, BVB_O:BVB_O + 256] = np.tile(
            np.asarray(bv, np.float32)[sl][None, :], (128, 1)
        )
        bpk_h = np.stack(
            [np.asarray(bb, np.float32)[sl][fc * 128:(fc + 1) * 128]
             for bb, fc in ((bq, 0), (bq, 1), (bk, 0), (bk, 1))],
            axis=1,
        )
        in_maps.append({
            "xtr": xtr_b[b],
            "wpk": np.ascontiguousarray(wpk_h).astype(bf),
            "bpk": np.ascontiguousarray(bpk_h),
        })

    trace = bool(int(os.environ.get("ATTN_KERNEL_TRACE", "0")))
    kw = {}
    if trace:
        tdir = os.environ.get("ATTN_KERNEL_TRACE_DIR")
        if tdir:
            os.makedirs(tdir, exist_ok=True)
            kw["tmpdir"] = tdir
    res = run_bass_kernel_spmd(nc, in_maps, core_ids=list(range(N_CORES)), trace=trace, **kw)
    if trace:
        print(f"HW exec time: {res.exec_time_ns} ns")
        _cache["exec_time_ns"] = res.exec_time_ns
        _cache["res"] = res

    out = np.zeros((B, S, DM), np.float32)
    for c in range(N_CORES):
        b = c // 4
        part = np.asarray(res.results[c]["out"]).astype(np.float32)
        out[b] += (
            part.reshape(128, NRC, 4, DM).transpose(1, 2, 0, 3).reshape(S, DM)
        )
    return out


# revision 4
# speedup vs baseline: 1.0093x; 1.0093x over previous
"""Distributed causal multi-head attention for Trainium2 (8 NeuronCores), v2.

Problem: B=2, S=2048, d_model=1024, 16 heads x 64 dims, causal softmax attention.

Strategy v2 (DP over batch x TP over 4 heads; host-side reduction):
  - Core c = (b, g): batch b = c//4, head-group g = c%4 owns heads 4g..4g+3
    (256 of the 1024 QKV features). Input per core: x[b] (4 MB bf16) +
    weights slice (2.2 MB) vs 9.6 MB in v1; output partial 4 MB vs 8 MB.
    DMA is the shared bottleneck (~120-150 GB/s/core under 8-core
    contention), so halving bytes shrinks the DMA-gated head and tail.
  - Within a core, the 4 heads are processed as 2 head-pairs p in {0,1}
    using the same S^T-layout attention machinery as v1: scores^T = K^T x Q^T
    with the pair's two K=64 matmuls row-tiled onto disjoint PE row groups,
    exp on ScalarE, causal triu mask multiply on the diagonal tiles, AV
    accumulation in PSUM with 64 appended ones columns per head producing the
    softmax denominator replicated in PSUM partitions 64-127 for free.
  - V projection computes all 4 heads at once (moving N=256 vs 128 in v1,
    halving its LDWEIGHTS-bound cost); output tile layout per k-tile is
    [v_h0|ones|v_h1|ones|v_h2|ones|v_h3|ones] written with strided APs
    (one bias-add + one memset per k-tile).
  - Chunk order (q0,p0),(q0,p1),(q1,p0),(q1,p1),... matches DMA arrival
    (chunk qc only needs x-chunks 0..qc); output projection of q-chunk qc
    runs as filler inside later chunks; the q3 output projection pipelines
    per-rt into the last chunk's normalization (short exposed tail).
  - Output is packed [128, 4*4096] (contiguous 8 KB per-partition rows ->
    efficient DMA packets); the host unpacks and sums the 4 partials/batch.
  - Dummy warmup matmuls keep the PE HAM activity monitor from throttling
    the clock during the DMA-gated ramp.
"""
import os
import sys

sys.path.insert(0, "/opt/trn_rl_repo")

import numpy as np
import ml_dtypes

from concourse import bacc, mybir, tile
from concourse.ap import AP
from concourse.bass_utils import run_bass_kernel_spmd

BF16 = mybir.dt.bfloat16
F32 = mybir.dt.float32

B, S, DM = 2, 2048, 1024
H, DK = 16, 64
N_CORES = 8
FPC = 256            # features per core = 4 heads x 64
NRC = 4              # 512-token chunks per batch
NKT = S // 128       # k-tiles per batch = 16
SCALE = 1.0 / 8.0    # 1/sqrt(64)
# wpk cols: wq_f0 (1024) | wk_f0 (1024) | wv (2048) | wq_f1 (1024) | wk_f1
#           (1024) | wo_p0 (1024) | wo_p1 (1024) | mask (128) | bvb (256)
# DMA pieces: [0:4096] (all chunk-A needs), [4096:6144], [6144:8576]
WQ0_O, WK0_O, WV_O, WQ1_O, WK1_O, WO_O = 0, 1024, 2048, 4096, 5120, 6144
MASK_O, BVB_O = 8192, 8320
WPK_N = 8576
N_DUMMY = 27

_cache = {}


def _blocked(ap_src, nblk, blk_stride, blk_w, col_off=0):
    """AP selecting nblk blocks of blk_w cols spaced blk_stride, from a 2D AP."""
    t = ap_src
    p = list(t.ap)
    # p = [[pstride, np], [1, ncols]]
    return AP(t.tensor, t.offset + col_off, [list(p[0]), [blk_stride, nblk], [1, blk_w]])


def _build():
    nc = bacc.Bacc("TRN2", target_bir_lowering=False, debug=False, num_devices=N_CORES)

    # xtr[p, rc*4096 + kc*512 + j] = x[b]^T[kc*128+p, rc*512+j]
    xtr = nc.dram_tensor("xtr", [128, NRC * 4096], BF16, kind="ExternalInput")
    wpk = nc.dram_tensor("wpk", [128, WPK_N], BF16, kind="ExternalInput")
    bpk = nc.dram_tensor("bpk", [128, 4], F32, kind="ExternalInput")
    # out[p, qc*4096 + rt*1024 + d] = partial_out[qc*512 + rt*128 + p, d]
    out_ext = nc.dram_tensor("out", [128, NRC * 4096], BF16, kind="ExternalOutput")

    EXP = mybir.ActivationFunctionType.Exp
    IDENT = mybir.ActivationFunctionType.Identity

    with tile.TileContext(nc) as tc:
        with (
            tc.tile_pool(name="xtp", bufs=1) as xtp,
            tc.tile_pool(name="wts", bufs=1) as wts,
            tc.tile_pool(name="qkv", bufs=1) as qkvp,
            tc.tile_pool(name="vnp", bufs=1) as vnp,
            tc.tile_pool(name="pp", bufs=4) as pp,
            tc.tile_pool(name="den", bufs=2) as denp,
            tc.tile_pool(name="otp", bufs=1) as otp,
            tc.tile_pool(name="outp", bufs=2) as outp,
            tc.tile_pool(name="psmm", bufs=2, space="PSUM") as psmm,
            tc.tile_pool(name="psS", bufs=2, space="PSUM") as psS,
            tc.tile_pool(name="psO", bufs=1, space="PSUM") as psO,
        ):
            # ---------- dummy warmup (PE busy during input DMA => HAM hot) ----
            dummy_sb = wts.tile([128, 512], BF16, tag="dum", name="dummy_sb")
            nc.vector.memset(dummy_sb[:], 0.0)

            def dummy(i, n=512):
                ps = psS.tile([128, 1024], F32, tag="s", name=f"dum{i}")
                nc.tensor.matmul(
                    ps[:, 0:n], dummy_sb[:, 0:128], dummy_sb[:, 0:n],
                    start=True, stop=True,
                )

            for i in range(N_DUMMY):
                dummy(i)

            # ---------- input DMAs, in consumption order ----------
            wpk_sb = wts.tile([128, WPK_N], BF16, tag="wpk", name="wpk_sb")
            xt_all = xtp.tile([128, NRC * 4096], BF16, tag="xt", name="xt_all")
            bpk_sb = wts.tile([128, 4], F32, tag="bpk", name="bpk_sb")
            nc.sync.dma_start(wpk_sb[:, 0:WQ1_O], wpk[:, 0:WQ1_O])  # qk f0 + wv
            nc.sync.dma_start(bpk_sb[:], bpk[:])
            nc.sync.dma_start(xt_all[:, 0:4096], xtr[:, 0:4096])    # rc0
            nc.sync.dma_start(wpk_sb[:, WQ1_O:WO_O], wpk[:, WQ1_O:WO_O])
            nc.sync.dma_start(
                xt_all[:, 4096:2 * 4096], xtr[:, 4096:2 * 4096]     # rc1
            )
            nc.sync.dma_start(wpk_sb[:, WO_O:WPK_N], wpk[:, WO_O:WPK_N])
            for rc in range(2, NRC):
                nc.sync.dma_start(
                    xt_all[:, rc * 4096:(rc + 1) * 4096],
                    xtr[:, rc * 4096:(rc + 1) * 4096],
                )

            _wbase = {("q", 0): WQ0_O, ("k", 0): WK0_O, ("q", 1): WQ1_O, ("k", 1): WK1_O}

            def wqk_sl(which, kc, fc):   # [128, 128] stationary
                o = _wbase[(which, fc)] + kc * 128
                return wpk_sb[:, o:o + 128]

            def wv_sl(kc):               # [128, 256] moving
                o = WV_O + kc * 256
                return wpk_sb[:, o:o + 256]

            wo_sb = [wpk_sb[:, WO_O:WO_O + DM], wpk_sb[:, WO_O + DM:WO_O + 2 * DM]]
            mask_sb = wpk_sb[:, MASK_O:MASK_O + 128]
            bvb_sb = wpk_sb[:, BVB_O:BVB_O + 256]
            b_col = {("q", 0): 0, ("q", 1): 1, ("k", 0): 2, ("k", 1): 3}

            # qT/kT per head-pair: [128 (2 heads x 64), 2048 tokens]
            qT = [qkvp.tile([128, S], BF16, tag=f"qT{p}", name=f"qT{p}") for p in (0, 1)]
            kT = [qkvp.tile([128, S], BF16, tag=f"kT{p}", name=f"kT{p}") for p in (0, 1)]
            # v natural layout per k-tile: [128 tok, v0|1s|v1|1s|v2|1s|v3|1s]
            v_nat = [None] * NKT
            ot_st = {}
            osb_st = {}

            def xsl(rc, kc, lo, n):
                o = rc * 4096 + kc * 512 + lo
                return xt_all[:, o:o + n]

            # ---------- filler units ----------
            def unit_proj(rc, which, fc):
                def emit():
                    ps = psmm.tile([128, 512], F32, tag="mm", name=f"ps_{which}{fc}_{rc}")
                    for kc in range(8):
                        nc.tensor.matmul(
                            ps[:], wqk_sl(which, kc, fc), xsl(rc, kc, 0, 512),
                            start=(kc == 0), stop=(kc == 7),
                        )
                    dst = (qT if which == "q" else kT)[fc]
                    nc.scalar.activation(
                        dst[:, rc * 512:(rc + 1) * 512], ps[:], IDENT,
                        bias=bpk_sb[:, b_col[(which, fc)]:b_col[(which, fc)] + 1],
                    )
                return emit

            def unit_vtr(rc, i):
                def emit():
                    kt = rc * 4 + i
                    ps = psmm.tile([128, 512], F32, tag="mm", name=f"psv{rc}_{i}")
                    for kc in range(8):
                        nc.tensor.matmul(
                            ps[:, 0:256], xsl(rc, kc, i * 128, 128), wv_sl(kc),
                            start=(kc == 0), stop=(kc == 7),
                        )
                    vn = vnp.tile([128, 512], BF16, tag=f"vn{kt}", name=f"vn{kt}")
                    # vn[:, 128h:128h+64] = ps[:, 64h:64h+64] + bvb[:, 64h:64h+64]
                    nc.vector.tensor_add(
                        _blocked(vn[:], 4, 128, 64),
                        _blocked(ps[:], 4, 64, 64),
                        bvb_sb[:, 0:256],
                    )
                    nc.vector.memset(_blocked(vn[:], 4, 128, 64, col_off=64), 1.0)
                    v_nat[kt] = vn
                return emit

            def unit_outproj(qc, rt, nci):
                def emit():
                    ps = psmm.tile([128, 512], F32, tag="mm", name=f"pso{qc}_{rt}_{nci}")
                    for p in (0, 1):
                        lh = ot_st[(p, qc)][:, rt * 128:(rt + 1) * 128]
                        nc.tensor.matmul(
                            ps[:], lh, wo_sb[p][:, nci * 512:(nci + 1) * 512],
                            start=(p == 0), stop=(p == 1),
                        )
                    osb = osb_st[qc]
                    dst = osb[:, rt * 1024 + nci * 512: rt * 1024 + (nci + 1) * 512]
                    if qc == 3 and nci == 0:
                        nc.scalar.copy(dst, ps[:])
                    else:
                        nc.vector.tensor_copy(dst, ps[:])
                    if rt == 3 and nci == 1 and qc < 3:
                        nc.sync.dma_start(
                            out_ext[:, qc * 4096:(qc + 1) * 4096], osb[:]
                        )
                return emit

            def new_osb(qc):
                osb = outp.tile([128, 4096], BF16, tag="ob", name=f"osb{qc}")
                osb_st[qc] = osb

            # ---------- attention chunk ----------
            def chunk(p, qc, fillers, pre_av=None, last=False):
                nkt = 4 * qc + 4
                q0 = qc * 512
                fillers = list(fillers)[::-1]
                n_fill = len(fillers)

                o_ps = [
                    psO.tile([128, 512], F32, tag=f"o{h}", name=f"o_ps{h}_{p}_{qc}")
                    for h in (0, 1)
                ]

                def emit_s(kt):
                    lo = max(0, 128 * (kt - 4 * qc))
                    s_ps = psS.tile([128, 1024], F32, tag="s", name=f"s_{p}_{qc}_{kt}")
                    k_sl = slice(kt * 128, (kt + 1) * 128)
                    for h in (0, 1):
                        hp = slice(64 * h, 64 * h + 64)
                        nc.tensor.matmul(
                            s_ps[:, 512 * h + lo:512 * h + 512],
                            kT[p][hp, k_sl], qT[p][hp, q0 + lo:q0 + 512],
                            start=True, stop=True,
                        )
                    return s_ps, lo

                def emit_exp_av(kt, s_ps, lo):
                    p_sb = pp.tile([128, 1024], BF16, tag="p", name=f"p_{p}_{qc}_{kt}")
                    if lo == 0:
                        nc.scalar.activation(p_sb[:], s_ps[:], EXP, scale=SCALE)
                    elif lo <= 256:
                        # one contiguous ACTIVATE spanning both heads' valid
                        # ranges plus the dead gap [512:512+lo] (written but
                        # never read by mask or AV): cheaper than two ops
                        # while 1024-lo < 2*(512-lo) + ACT fixed cost
                        nc.scalar.activation(
                            p_sb[:, lo:1024], s_ps[:, lo:1024], EXP, scale=SCALE
                        )
                    else:
                        for h in (0, 1):
                            nc.scalar.activation(
                                p_sb[:, 512 * h + lo:512 * h + 512],
                                s_ps[:, 512 * h + lo:512 * h + 512],
                                EXP, scale=SCALE,
                            )
                    d = 128 * (kt - 4 * qc)
                    if d >= 0:
                        hi = min(512, d + 128)
                        for h in (0, 1):
                            nc.vector.tensor_mul(
                                p_sb[:, 512 * h + lo:512 * h + hi],
                                p_sb[:, 512 * h + lo:512 * h + hi],
                                mask_sb[:, 0:hi - lo],
                            )
                    for h in (0, 1):
                        nc.tensor.matmul(
                            o_ps[h][:, lo:512],
                            v_nat[kt][:, 128 * (2 * p + h):128 * (2 * p + h) + 128],
                            p_sb[:, 512 * h + lo:512 * h + 512],
                            start=(kt == 0), stop=(kt == nkt - 1),
                        )

                popped = 0
                s_cur = None
                for kt in range(nkt):
                    if pre_av is not None and kt in pre_av:
                        if s_cur is None:
                            s_cur = emit_s(kt)
                        pre_av[kt]()
                    if s_cur is None:
                        s_cur = emit_s(kt)
                    s_nxt = emit_s(kt + 1) if kt + 1 < nkt else None
                    # one filler early: covers the exp latency of the first
                    # k-tile before any AV work is ready for the PE
                    if kt == 0 and pre_av is None and fillers:
                        fillers.pop()()
                        popped += 1
                    emit_exp_av(kt, *s_cur)
                    s_cur = s_nxt
                    spread = nkt + 4 if last else nkt
                    want = (kt + 1) * n_fill // spread
                    while fillers and popped < want:
                        fillers.pop()()
                        popped += 1

                # ---- normalization (denominator in psum partitions 64-127) --
                ot = otp.tile([128, 512], BF16, tag=f"ot{p}_{qc}", name=f"ot{p}_{qc}")
                ot_st[(p, qc)] = ot
                rcp = [None, None]
                if not last:
                    for h in (0, 1):
                        rcp[h] = denp.tile([64, 512], F32, tag=f"d{h}", name=f"d{h}_{p}{qc}")
                        nc.vector.tensor_copy(rcp[h][0:64, :], o_ps[h][64:128, :])
                        nc.vector.reciprocal_approx_fast(rcp[h][0:64, :], rcp[h][0:64, :])
                        nc.vector.tensor_mul(
                            ot[64 * h:64 * h + 64, :], o_ps[h][0:64, :], rcp[h][0:64, :]
                        )
                    while fillers:
                        fillers.pop()()
                else:
                    # last chunk: per-rt normalization pipelined into the q3
                    # output projection, with dummies keeping the PE warm.
                    new_osb(3)
                    td = [0]

                    def tail_dummy(n=256):
                        dummy(f"t{td[0]}", n=n)
                        td[0] += 1

                    for h in (0, 1):
                        rcp[h] = denp.tile([64, 512], F32, tag=f"d{h}", name=f"d{h}_t")
                    nc.scalar.copy(rcp[0][0:64, :], o_ps[0][64:128, :])
                    nc.vector.tensor_copy(rcp[1][0:64, :], o_ps[1][64:128, :])
                    for h in (0, 1):
                        nc.vector.reciprocal_approx_fast(rcp[h][0:64, :], rcp[h][0:64, :])
                    tail_dummy(512)
                    tail_dummy(512)
                    for rt in range(4):
                        csl = slice(rt * 128, (rt + 1) * 128)
                        for h in (0, 1):
                            nc.vector.tensor_mul(
                                ot[64 * h:64 * h + 64, csl],
                                o_ps[h][0:64, csl],
                                rcp[h][0:64, csl],
                            )
                        tail_dummy()
                        for nci in (0, 1):
                            unit_outproj(3, rt, nci)()
                            nc.sync.dma_start(
                                out_ext[:, 3 * 4096 + rt * 1024 + nci * 512:
                                        3 * 4096 + rt * 1024 + (nci + 1) * 512],
                                osb_st[3][:, rt * 1024 + nci * 512:
                                        rt * 1024 + (nci + 1) * 512],
                            )
                        if rt < 3:
                            tail_dummy()

            # ---------- schedule ----------
            # pre-A units: rc0 projections for pair 0 + V k-tiles 0..3
            unit_proj(0, "q", 0)()
            unit_proj(0, "k", 0)()

            # A = (q0, p0): V(0, i) interleaved before each AV
            chunk(0, 0,
                  fillers=[unit_proj(0, "k", 1), unit_proj(0, "q", 1)],
                  pre_av={i: unit_vtr(0, i) for i in range(4)})
            # B = (q0, p1)
            chunk(1, 0, fillers=[unit_proj(1, "q", 0), unit_proj(1, "k", 0)]
                  + [unit_vtr(1, i) for i in range(4)])
            new_osb(0)
            # C = (q1, p0)
            chunk(0, 1, fillers=[unit_proj(1, "q", 1), unit_proj(1, "k", 1)]
                  + [unit_outproj(0, rt, nci) for rt in range(4) for nci in (0, 1)])
            # D = (q1, p1)
            chunk(1, 1, fillers=[unit_proj(2, "q", 0), unit_proj(2, "k", 0)]
                  + [unit_vtr(2, i) for i in range(4)])
            new_osb(1)
            # E = (q2, p0)
            chunk(0, 2, fillers=[unit_proj(2, "q", 1), unit_proj(2, "k", 1)]
                  + [unit_outproj(1, rt, nci) for rt in range(4) for nci in (0, 1)])
            # F = (q2, p1)
            chunk(1, 2, fillers=[unit_proj(3, "q", 0), unit_proj(3, "k", 0)]
                  + [unit_vtr(3, i) for i in range(4)])
            new_osb(2)
            # G = (q3, p0)
            chunk(0, 3, fillers=[unit_proj(3, "q", 1), unit_proj(3, "k", 1)]
                  + [unit_outproj(2, rt, nci) for rt in range(4) for nci in (0, 1)])
            # H = (q3, p1): last; outproj(3) pipelined per-rt inside
            chunk(1, 3, fillers=[], last=True)

    nc.compile()
    return nc


def kernel(x, Wq, bq, Wk, bk, Wv, bv, Wo):
    if "nc" not in _cache:
        _cache["nc"] = _build()
    nc = _cache["nc"]

    bf = ml_dtypes.bfloat16
    x_f = np.asarray(x, np.float32)
    wo_f = np.asarray(Wo, np.float32)
    trimask = np.triu(np.ones((128, 128), np.float32))

    xtr_b = []
    for b in range(B):
        xT = x_f[b].T                                     # [1024, 2048]
        xtr_b.append(np.ascontiguousarray(
            xT.reshape(8, 128, NRC, 512).transpose(1, 2, 0, 3).reshape(128, NRC * 4096)
        ).astype(bf))

    in_maps = []
    for c in range(N_CORES):
        b, g = c // 4, c % 4
        sl = slice(g * FPC, (g + 1) * FPC)
        wpk_h = np.empty((128, WPK_N), np.float32)
        for (base0, base1), W in (((WQ0_O, WQ1_O), Wq), ((WK0_O, WK1_O), Wk)):
            Wc = np.asarray(W, np.float32)[:, sl]         # [1024, 256]
            for fc, base in ((0, base0), (1, base1)):
                wpk_h[:, base:base + 1024] = (
                    Wc[:, fc * 128:(fc + 1) * 128]
                    .reshape(8, 128, 128).transpose(1, 0, 2).reshape(128, 1024)
                )
        Wvc = np.asarray(Wv, np.float32)[:, sl]
        wpk_h[:, WV_O:WV_O + 2048] = (
            Wvc.reshape(8, 128, 256).transpose(1, 0, 2).reshape(128, 2048)
        )
        wpk_h[:, WO_O:WO_O + DM] = wo_f[sl][0:128, :]
        wpk_h[:, WO_O + DM:WO_O + 2 * DM] = wo_f[sl][128:256, :]
        wpk_h[:, MASK_O:MASK_O + 128] = trimask
        wpk_h[:, BVB_O:BVB_O + 256] = np.tile(
            np.asarray(bv, np.float32)[sl][None, :], (128, 1)
        )
        bpk_h = np.stack(
            [np.asarray(bb, np.float32)[sl][fc * 128:(fc + 1) * 128]
             for bb, fc in ((bq, 0), (bq, 1), (bk, 0), (bk, 1))],
            axis=1,
        )
        in_maps.append({
            "xtr": xtr_b[b],
            "wpk": np.ascontiguousarray(wpk_h).astype(bf),
            "bpk": np.ascontiguousarray(bpk_h),
        })

    trace = bool(int(os.environ.get("ATTN_KERNEL_TRACE", "0")))
    kw = {}
    if trace:
        tdir = os.environ.get("ATTN_KERNEL_TRACE_DIR")
        if tdir:
            os.makedirs(tdir, exist_ok=True)
            kw["tmpdir"] = tdir
    res = run_bass_kernel_spmd(nc, in_maps, core_ids=list(range(N_CORES)), trace=trace, **kw)
    if trace:
        print(f"HW exec time: {res.exec_time_ns} ns")
        _cache["exec_time_ns"] = res.exec_time_ns
        _cache["res"] = res

    out = np.zeros((B, S, DM), np.float32)
    for c in range(N_CORES):
        b = c // 4
        part = np.asarray(res.results[c]["out"]).astype(np.float32)
        out[b] += (
            part.reshape(128, NRC, 4, DM).transpose(1, 2, 0, 3).reshape(S, DM)
        )
    return out
